# revision 1
# baseline (speedup 1.0000x reference)
"""2-layer GAT (PyG GATConv semantics) on 8 Trainium2 NeuronCores via Bass/Tile.

Strategy (dst-owner node sharding):
  - Nodes padded to N_pad = 8*npc, core k owns nodes [k*npc, (k+1)*npc).
  - Edges (+self loops) routed to their dst-owner core, grouped by 128-node
    dst block, chunked into 128-edge chunks (half-pure for int16 gather idx).
  - Phase A (replicated): hes1[n] = [x@W1 (c,h interleaved) | es1 f32] for all
    nodes in a per-core permuted order (own nodes first within each table
    half); per-node ed1 extracted to a small gatherable table.
  - Phase B: per chunk, dma_gather 768B rows by src + 256B ed rows by dst,
    p = exp(leakyrelu(es+ed)), one-hot matmul aggregation into PSUM
    (lhsT = M[e,i], rhs = p-scaled h slices + p for softmax denominators),
    per-block normalize + bias + ELU -> h1 (SBUF resident).
  - Phase C: h1 @ W2ext per own block (PE transpose + matmul) -> hes2loc,
    AllGather -> hes2 (global order), ed2 table.
  - Phase D/E: layer-2 edge phase (1 head) + y = h2@Wo + bo per block.
Host does only integer schedule construction, layout packing and weight
folding (a_src/a_dst folded into extra W columns).
"""
import sys

sys.path.insert(0, "/opt/trn_rl_repo")

import contextlib
import dataclasses
import math

import numpy as np

import concourse.bass as bass
import concourse.tile as tile
from concourse import bacc, mybir
from concourse.masks import make_identity
from concourse.bass_utils import run_bass_kernel_spmd

BLK = 128
NEG_SLOPE = 0.2
F32 = mybir.dt.float32
BF = mybir.dt.bfloat16
I16 = mybir.dt.int16
I32 = mybir.dt.int32


def _rep(ap, pattern, offset=None):
    new = dataclasses.replace(ap, ap=pattern)
    if offset is not None:
        new = dataclasses.replace(new, offset=offset)
    return new


# ----------------------------------------------------------------------------
# host-side schedule
# ----------------------------------------------------------------------------

def _make_schedule(edge_index, N, ncores, in_ch, heads, hid):
    bpc = math.ceil(N / (ncores * BLK))
    npc = bpc * BLK
    n_pad = npc * ncores
    half = n_pad // 2

    # per-core permutation: own nodes first within their half; half-preserving
    perms = []
    inv_perms = []
    for k in range(ncores):
        own = np.arange(k * npc, (k + 1) * npc, dtype=np.int64)
        alln = np.arange(n_pad, dtype=np.int64)
        others = alln[(alln < k * npc) | (alln >= (k + 1) * npc)]
        if ncores == 1:
            perm = alln
        elif (k + 1) * npc <= half:  # own range entirely in half A
            a = np.concatenate([own, others[others < half]])
            b = others[others >= half]
            perm = np.concatenate([a, b])
        else:  # own range entirely in half B
            a = others[others < half]
            b = np.concatenate([own, others[others >= half]])
            perm = np.concatenate([a, b])
        inv = np.empty(n_pad, dtype=np.int64)
        inv[perm] = alln
        perms.append(perm)
        inv_perms.append(inv)

    src = np.concatenate([edge_index[0].astype(np.int64),
                          np.arange(n_pad, dtype=np.int64)])
    dst = np.concatenate([edge_index[1].astype(np.int64),
                          np.arange(n_pad, dtype=np.int64)])
    owner = dst // npc

    # per-core edge lists grouped by (block, half)
    per_core = []
    cnt = np.zeros((ncores, bpc, 2), np.int64)
    for k in range(ncores):
        m = owner == k
        s_g = src[m]
        dloc = dst[m] - k * npc
        h = (s_g >= half).astype(np.int64) if ncores > 1 else np.zeros_like(s_g)
        b = dloc // BLK
        order = np.lexsort((h, b))
        s_g, dloc, h, b = s_g[order], dloc[order], h[order], b[order]
        per_core.append((s_g, dloc, h, b))
        np.add.at(cnt[k], (b, h), 1)

    nhalves = 2 if ncores > 1 else 1
    chunks_bh = np.ceil(cnt.max(axis=0) / 128).astype(np.int64)  # [bpc, 2]
    slots = []  # (block, half, first_of_block, last_of_block)
    for b in range(bpc):
        tot = int(chunks_bh[b, :nhalves].sum())
        assert tot >= 1
        i = 0
        for h in range(nhalves):
            for _ in range(int(chunks_bh[b, h])):
                slots.append((b, h, i == 0, i == tot - 1))
                i += 1
    nch = len(slots)

    # per-core packed index arrays
    def wrap16(vals):  # vals [nch, 128] -> [128, nch*8] int16
        v = vals.reshape(nch, 8, 16)
        base = np.zeros((16, nch * 8), np.int16)
        for ci in range(nch):
            base[:, ci * 8:(ci + 1) * 8] = v[ci].T
        out = np.zeros((128, nch * 8), np.int16)
        for r in range(8):
            out[r * 16:(r + 1) * 16] = base
        return out

    idx1 = np.zeros((ncores, 128, nch * 8), np.int16)
    idx2 = np.zeros((ncores, 128, nch * 8), np.int16)
    idxe1 = np.zeros((ncores, 128, nch * 8), np.int16)
    idxe2 = np.zeros((ncores, 128, nch * 8), np.int16)
    dstl = np.full((ncores, 128, nch), -1.0, np.float32)
    for k in range(ncores):
        s_g, dloc, h, b = per_core[k]
        p_src = inv_perms[k][s_g]
        v1 = np.zeros((nch, 128), np.int64)
        v2 = np.zeros((nch, 128), np.int64)
        ve1 = np.zeros((nch, 128), np.int64)
        ve2 = np.zeros((nch, 128), np.int64)
        vd = np.full((nch, 128), -1.0, np.float32)
        # bucket boundaries per (b, h)
        keys = b * 2 + h
        ptr = 0
        ci = 0
        for bb in range(bpc):
            for hh in range(nhalves):
                n = int(cnt[k, bb, hh])
                sl = slice(ptr, ptr + n)
                e_p = p_src[sl] - hh * half
                e_g = s_g[sl] - hh * half
                e_d = dloc[sl] + (npc if (ncores > 1 and k >= ncores // 2) else 0)
                e_dl = (dloc[sl] - bb * BLK).astype(np.float32)
                e_d2 = dloc[sl]
                ptr += n
                ncb = int(chunks_bh[bb, hh])
                for c in range(ncb):
                    a0, a1 = c * 128, min((c + 1) * 128, n)
                    ln = max(0, a1 - a0)
                    if ln > 0:
                        v1[ci, :ln] = e_p[a0:a0 + ln]
                        v2[ci, :ln] = e_g[a0:a0 + ln]
                        ve1[ci, :ln] = e_d[a0:a0 + ln]
                        ve2[ci, :ln] = e_d2[a0:a0 + ln]
                        vd[ci, :ln] = e_dl[a0:a0 + ln]
                    ci += 1
        assert ci == nch and ptr == len(s_g)
        idx1[k] = wrap16(v1)
        idx2[k] = wrap16(v2)
        idxe1[k] = wrap16(ve1)
        idxe2[k] = wrap16(ve2)
        dstl[k] = vd.T

    return dict(
        ncores=ncores, bpc=bpc, npc=npc, n_pad=n_pad, half=half,
        nhalves=nhalves, slots=slots, nch=nch,
        perms=perms, idx1=idx1, idx2=idx2, idxe1=idxe1, idxe2=idxe2,
        dstl=dstl,
    )


# ----------------------------------------------------------------------------
# device program
# ----------------------------------------------------------------------------

def _build_program(cfg, heads, hid, in_ch, bias_nonzero, bo_val, dbg=False,
                   sim_nocc=False, lim_blocks=None, strip=0, reps=1):
    ncores = cfg["ncores"]
    bpc, npc, n_pad, half = cfg["bpc"], cfg["npc"], cfg["n_pad"], cfg["half"]
    slots, nch = cfg["slots"], cfg["nch"]
    HC = heads * hid  # 256
    R1 = 384           # hes1 row elems (bf16): h 0:256, es f32 slots 256:264
    R2 = 128           # hes2 row elems: h2 0:64, es2 f32 slots 64:66
    RE = 128           # ed table row elems
    W1C = HC + 2 * heads  # 264
    W2C = hid + 2          # 66
    ntile = n_pad // BLK

    nc = bacc.Bacc("TRN2", target_bir_lowering=False, debug=False,
                   enable_asserts=True, num_devices=ncores,
                   num_swdge_queues=1, dynamic_dma_scratch_size=65536)

    xT = nc.dram_tensor("xT", [in_ch, n_pad], BF, kind="ExternalInput").ap()
    w1e = nc.dram_tensor("w1e", [in_ch, W1C], BF, kind="ExternalInput").ap()
    w2e = nc.dram_tensor("w2e", [HC, W2C], BF, kind="ExternalInput").ap()
    b1r = nc.dram_tensor("b1r", [1, HC], BF, kind="ExternalInput").ap()
    b2r = nc.dram_tensor("b2r", [1, hid], BF, kind="ExternalInput").ap()
    wor = nc.dram_tensor("wor", [1, hid], BF, kind="ExternalInput").ap()
    idx1 = nc.dram_tensor("idx1", [128, nch * 8], I16, kind="ExternalInput").ap()
    idx2 = nc.dram_tensor("idx2", [128, nch * 8], I16, kind="ExternalInput").ap()
    idxe1 = nc.dram_tensor("idxe1", [128, nch * 8], I16, kind="ExternalInput").ap()
    idxe2 = nc.dram_tensor("idxe2", [128, nch * 8], I16, kind="ExternalInput").ap()
    dstl = nc.dram_tensor("dstl", [128, nch], F32, kind="ExternalInput").ap()
    yout = nc.dram_tensor("y", [128, bpc], F32, kind="ExternalOutput").ap()

    hes1 = nc.dram_tensor("hes1", [n_pad, R1], BF).ap()
    h1dbg = (nc.dram_tensor("h1dbg", [128, bpc * HC], BF,
                            kind="ExternalOutput").ap() if dbg else None)
    sdbg = (nc.dram_tensor("sdbg", [128, bpc * heads], F32,
                           kind="ExternalOutput").ap() if dbg else None)
    aggdbg = (nc.dram_tensor("aggdbg", [128, HC], F32,
                             kind="ExternalOutput").ap() if dbg else None)
    pdbg = (nc.dram_tensor("pdbg", [128, 64], F32,
                           kind="ExternalOutput").ap() if dbg else None)
    scdbg = (nc.dram_tensor("scdbg", [128, HC], F32,
                            kind="ExternalOutput").ap() if dbg else None)
    ned1 = npc * (2 if ncores > 1 else 1)
    ed1t = nc.dram_tensor("ed1t", [ned1, RE], BF).ap()
    ed2t = nc.dram_tensor("ed2t", [npc, RE], BF).ap()
    hes2loc = nc.dram_tensor("hes2loc", [npc, R2], BF).ap()
    if ncores > 1:
        hes2 = nc.dram_tensor("hes2", [n_pad, R2], BF,
                              addr_space="Shared").ap()
    else:
        hes2 = nc.dram_tensor("hes2", [n_pad, R2], BF).ap()

    # own tile ranges in phase A (permuted order)
    if ncores == 1:
        own_ranges = [(0, bpc, 0)]
    else:
        own_ranges = [(0, bpc, 0), (half // BLK, half // BLK + bpc, npc)]

    def own_row_off(t):
        for lo, hi, off in own_ranges:
            if lo <= t < hi:
                return off + (t - lo) * BLK
        return None

    with tile.TileContext(nc) as tc, contextlib.ExitStack() as _stk:
        consts = _stk.enter_context(tc.tile_pool(name="consts", bufs=1))
        iota_i = consts.tile([128, 128], I32)
        nc.gpsimd.iota(iota_i[:], pattern=[[1, 128]], base=0, channel_multiplier=0)
        iota_bf = consts.tile([128, 128], BF)
        nc.vector.tensor_copy(iota_bf[:], iota_i[:])
        ident = consts.tile([128, 128], BF)
        make_identity(nc, ident[:])
        w1t = consts.tile([128, W1C], BF)
        nc.sync.dma_start(w1t[:], w1e[:])
        w2t = consts.tile([128, 2, W2C], BF)
        nc.sync.dma_start(w2t[:, 0, :], w2e[0:128, :])
        nc.sync.dma_start(w2t[:, 1, :], w2e[128:256, :])
        b1t = consts.tile([128, HC], BF)
        b2t = consts.tile([128, hid], BF)
        wot = consts.tile([128, hid], BF)
        onecol = consts.tile([1, 128], BF)
        nc.gpsimd.memset(onecol[:], 1.0)
        b1row = consts.tile([1, HC], BF)
        nc.sync.dma_start(b1row[:], b1r[:])
        b2row = consts.tile([1, hid], BF)
        nc.sync.dma_start(b2row[:], b2r[:])
        worow = consts.tile([1, hid], BF)
        nc.sync.dma_start(worow[:], wor[:])
        idx1t = consts.tile([128, nch * 8], I16)
        nc.sync.dma_start(idx1t[:], idx1[:])
        idxe1t = consts.tile([128, nch * 8], I16)
        nc.sync.dma_start(idxe1t[:], idxe1[:])
        dstlt = consts.tile([128, nch], F32)
        nc.sync.dma_start(dstlt[:], dstl[:])
        h1sb = consts.tile([128, bpc * HC], BF)
        ed2sb = consts.tile([128, bpc], BF)
        yall = consts.tile([128, bpc], F32)
        if lim_blocks is not None or strip > 0:
            nc.vector.memset(yall[:], 0.0)
            nc.vector.memset(h1sb[:], 0.0)
        if dbg:
            sdbg_sb = consts.tile([128, bpc * heads], F32)
            pdbg_sb = consts.tile([128, 64], F32)
            scdbg_sb = consts.tile([128, HC], F32)

        with tc.tile_pool(name="bcp", bufs=1, space="PSUM") as bcp:
            bps = bcp.tile([128, HC], F32)
            nc.tensor.matmul(bps[:, 0:HC], lhsT=onecol[:], rhs=b1row[:],
                             start=True, stop=True)
            nc.vector.tensor_copy(b1t[:], bps[:, 0:HC])
            bps2 = bcp.tile([128, HC], F32, tag="bps")
            nc.tensor.matmul(bps2[:, 0:hid], lhsT=onecol[:], rhs=b2row[:],
                             start=True, stop=True)
            nc.vector.tensor_copy(b2t[:], bps2[:, 0:hid])
            bps3 = bcp.tile([128, HC], F32, tag="bps")
            nc.tensor.matmul(bps3[:, 0:hid], lhsT=onecol[:], rhs=worow[:],
                             start=True, stop=True)
            nc.vector.tensor_copy(wot[:], bps3[:, 0:hid])

        # ---------------- phase A: hes1 = [x@W1 | es1], ed1 ----------------
        AT = 4
        RW = HC + 2 * heads
        with tc.tile_pool(name="pha", bufs=3) as pa, \
             tc.tile_pool(name="phaps", bufs=4, space="PSUM") as paps, \
             tc.tile_pool(name="phaed", bufs=2) as paed:
            for t0 in range(0, ntile, AT):
                tn = min(AT, ntile - t0)
                xt = pa.tile([128, tn * BLK], BF, tag="xt")
                nc.sync.dma_start(xt[:], xT[:, t0 * BLK:(t0 + tn) * BLK])
                row4 = pa.tile([128, tn, RW], BF, tag="row")
                for j in range(tn):
                    t = t0 + j
                    ps = paps.tile([128, W1C], F32, tag="ps")
                    nc.tensor.matmul(ps[:], lhsT=xt[:, j * BLK:(j + 1) * BLK],
                                     rhs=w1t[:], start=True, stop=True)
                    nc.scalar.copy(
                        _rep(row4[:], [[tn * RW, 128], [1, HC]], offset=j * RW),
                        ps[:, 0:HC])
                    nc.vector.tensor_copy(
                        _rep(row4[:], [[tn * RW, 128], [1, 2 * heads]],
                             offset=j * RW + HC).bitcast(F32),
                        ps[:, HC:HC + heads])
                    off = own_row_off(t)
                    if off is not None:
                        eds = paed.tile([128, RE], BF, tag="eds")
                        nc.vector.memset(eds[:], 0.0)
                        nc.vector.tensor_copy(eds[:, 0:heads],
                                              ps[:, HC + heads:HC + 2 * heads])
                        nc.sync.dma_start(ed1t[off:off + BLK, :], eds[:])
                out_ap = _rep(hes1[:],
                              [[R1, BLK], [BLK * R1, tn], [1, RW]],
                              offset=t0 * BLK * R1)
                nc.sync.dma_start(out_ap, row4[:])

        for _repeat_i in range(reps):
            # ---------------- phases B+C per own block ----------------
            GG = 8
            lim = bpc if lim_blocks is None else lim_blocks
            used = [ci for ci in range(nch) if slots[ci][0] < lim]
            groups = [used[g0:g0 + GG] for g0 in range(0, len(used), GG)]

            with tc.tile_pool(name="phb", bufs=4) as pb, \
                 tc.tile_pool(name="phbm", bufs=4) as pbm, \
                 tc.tile_pool(name="phbps", bufs=2, space="PSUM") as pbps, \
                 tc.tile_pool(name="phc", bufs=2) as pc, \
                 tc.tile_pool(name="phcps", bufs=2, space="PSUM") as pcps:
                blkps = None
                for grp in groups:
                    gn = len(grp)
                    g0 = grp[0]
                    assert grp == list(range(g0, g0 + gn))
                    gsup = pb.tile([128, gn, R1], BF, tag="gsup")
                    edg = pb.tile([128, gn, RE], BF, tag="edg")
                    # half-pure sub-ranges within the group share one gather
                    r0 = 0
                    while r0 < gn:
                        h = slots[grp[r0]][1]
                        r1 = r0
                        while r1 < gn and slots[grp[r1]][1] == h:
                            r1 += 1
                        tbl1 = (hes1[:] if cfg["nhalves"] == 1
                                else hes1[h * half:h * half + half, :])
                        nidx = (r1 - r0) * 128
                        nc.gpsimd.dma_gather(
                            out_ap=gsup[:, r0:r1, :],
                            in_ap=tbl1,
                            idxs_ap=idx1t[:, (g0 + r0) * 8:(g0 + r1) * 8],
                            num_idxs=nidx, num_idxs_reg=nidx, elem_size=R1)
                        r0 = r1
                    nc.gpsimd.dma_gather(
                        out_ap=edg[:],
                        in_ap=ed1t[:],
                        idxs_ap=idxe1t[:, g0 * 8:(g0 + gn) * 8],
                        num_idxs=gn * 128, num_idxs_reg=gn * 128, elem_size=RE)
                    if strip >= 3:
                        continue
                    # p = exp(lrelu(es + ed)) for the whole group
                    es_ap = gsup[:, :, HC:HC + 2 * heads].bitcast(F32)
                    ed_ap = edg[:, :, 0:heads]
                    lt = pbm.tile([128, gn * heads], F32, tag="lt")
                    lt3 = _rep(lt[:], [[gn * heads, 128], [heads, gn], [1, heads]])
                    nc.vector.tensor_tensor(out=lt3, in0=es_ap, in1=ed_ap,
                                            op=mybir.AluOpType.add)
                    lr = pbm.tile([128, gn * heads], F32, tag="lr")
                    nc.vector.tensor_scalar(out=lr[:], in0=lt[:],
                                            scalar1=NEG_SLOPE, scalar2=None,
                                            op0=mybir.AluOpType.mult)
                    nc.vector.tensor_tensor(out=lr[:], in0=lt[:], in1=lr[:],
                                            op=mybir.AluOpType.max)
                    ptf = pbm.tile([128, gn * heads], F32, tag="ptf")
                    nc.scalar.activation(ptf[:], lr[:],
                                         mybir.ActivationFunctionType.Exp)
                    ptb = pbm.tile([128, gn * heads], BF, tag="ptb")
                    nc.vector.tensor_copy(ptb[:], ptf[:])
                    if dbg and grp[0] == 0:
                        nc.vector.tensor_copy(pdbg_sb[:, 0:gn * heads], ptf[:])
                    if strip >= 2:
                        continue
                    for i, ci in enumerate(grp):
                        b, h, first, last = slots[ci]
                        if first:
                            blkps = pbps.tile([128, HC + heads], F32, tag="blkps")
                        m = pbm.tile([128, 128], BF, tag="m")
                        nc.vector.tensor_scalar(
                            out=m[:], in0=iota_bf[:],
                            scalar1=dstlt[:, ci:ci + 1], scalar2=None,
                            op0=mybir.AluOpType.is_equal)
                        # scale gathered h rows by p (interleaved (c,h) layout)
                        gslice = _rep(gsup[:],
                                      [[gn * R1, 128], [heads, hid], [1, heads]],
                                      offset=i * R1)
                        pbc = _rep(ptb[:], [[gn * heads, 128], [0, hid], [1, heads]],
                                   offset=i * heads)
                        nc.vector.tensor_tensor(out=gslice, in0=gslice, in1=pbc,
                                                op=mybir.AluOpType.mult)
                        if dbg and ci == 0:
                            nc.vector.tensor_copy(
                                scdbg_sb[:],
                                _rep(gsup[:], [[gn * R1, 128], [1, HC]], offset=0))
                        rhs = _rep(gsup[:], [[gn * R1, 128], [1, HC]],
                                   offset=i * R1)
                        nc.tensor.matmul(
                            blkps[:, 0:HC], lhsT=m[:], rhs=rhs,
                            start=first, stop=last, skip_group_check=True)
                        nc.tensor.matmul(
                            blkps[:, HC:HC + heads], lhsT=m[:],
                            rhs=ptb[:, i * heads:(i + 1) * heads],
                            start=False, stop=last, skip_group_check=True)

                        if last and strip >= 1:
                            continue
                        if last:
                            # -------- block drain: normalize, bias, ELU --------
                            if dbg:
                                nc.vector.tensor_copy(
                                    sdbg_sb[:, b * heads:(b + 1) * heads],
                                    blkps[:, HC:HC + heads])
                                if b == 0:
                                    nc.vector.tensor_copy(pdbg_sb[:, 32:64],
                                                          blkps[:, 0:32])
                            rs = pbm.tile([128, heads], F32, tag="rs")
                            nc.vector.reciprocal(rs[:], blkps[:, HC:HC + heads])
                            h1b = pbm.tile([128, HC], BF, tag="h1b")
                            ps3 = _rep(blkps[:],
                                       [[HC + heads, 128], [heads, hid], [1, heads]])
                            rsb = _rep(rs[:], [[heads, 128], [0, hid], [1, heads]])
                            h1b3 = _rep(h1b[:], [[HC, 128], [heads, hid], [1, heads]])
                            nc.vector.tensor_tensor(out=h1b3, in0=ps3, in1=rsb,
                                                    op=mybir.AluOpType.mult)
                            if bias_nonzero[0]:
                                nc.vector.tensor_tensor(
                                    out=h1b[:], in0=h1b[:], in1=b1t[:],
                                    op=mybir.AluOpType.add)
                            # elu = max(x,0) + min(exp(x)-1, 0)
                            ex = pbm.tile([128, HC], BF, tag="ex")
                            nc.scalar.activation(ex[:], h1b[:],
                                                 mybir.ActivationFunctionType.Exp)
                            nc.vector.tensor_scalar(
                                out=ex[:], in0=ex[:], scalar1=-1.0, scalar2=0.0,
                                op0=mybir.AluOpType.add, op1=mybir.AluOpType.min)
                            h1f = h1sb[:, b * HC:(b + 1) * HC]
                            nc.vector.tensor_scalar(
                                out=h1f, in0=h1b[:], scalar1=0.0, scalar2=None,
                                op0=mybir.AluOpType.max)
                            nc.vector.tensor_tensor(out=h1f, in0=h1f, in1=ex[:],
                                                    op=mybir.AluOpType.add)

                            # -------- phase C for this block --------
                            if strip == 4:
                                continue
                            ps2 = pcps.tile([128, W2C], F32, tag="ps2")
                            for kk in range(2):
                                trp = pcps.tile([128, 128], BF, tag="trp")
                                nc.tensor.transpose(
                                    trp[:],
                                    h1sb[:, b * HC + kk * 128: b * HC + kk * 128 + 128],
                                    ident[:])
                                h1T = pc.tile([128, 128], BF, tag="h1T")
                                nc.vector.tensor_copy(h1T[:], trp[:])
                                nc.tensor.matmul(ps2[:], lhsT=h1T[:],
                                                 rhs=w2t[:, kk, :],
                                                 start=kk == 0, stop=kk == 1)
                            h2row = pc.tile([128, R2], BF, tag="h2row")
                            nc.vector.memset(h2row[:], 0.0)
                            nc.vector.tensor_copy(h2row[:, 0:hid], ps2[:, 0:hid])
                            nc.vector.tensor_copy(
                                h2row[:, hid:hid + 2].bitcast(F32),
                                ps2[:, hid:hid + 1])
                            nc.sync.dma_start(hes2loc[b * BLK:(b + 1) * BLK, :],
                                              h2row[:])
                            ed2s = pc.tile([128, RE], BF, tag="ed2s")
                            nc.vector.memset(ed2s[:], 0.0)
                            nc.vector.tensor_copy(ed2s[:, 0:1],
                                                  ps2[:, hid + 1:hid + 2])
                            nc.sync.dma_start(ed2t[b * BLK:(b + 1) * BLK, :],
                                              ed2s[:])

            # ---------------- AllGather ----------------
            if strip >= 1:
                nc.vector.memset(h1sb[:], 0.0)
            if ncores > 1 and not sim_nocc:
                nc.gpsimd.collective_compute(
                    "AllGather", mybir.AluOpType.bypass,
                    replica_groups=[list(range(ncores))],
                    ins=[hes2loc[:]], outs=[hes2[:]])
            elif ncores == 1:
                nc.sync.dma_start(hes2[:], hes2loc[:])
            else:  # sim_nocc stand-in for AllGather: same local write volume
                for r in range(ncores):
                    nc.sync.dma_start(hes2[r * npc:(r + 1) * npc, :], hes2loc[:])

            # ---------------- phases D+E ----------------
            with tc.tile_pool(name="phd", bufs=4) as pd, \
                 tc.tile_pool(name="phdm", bufs=4) as pdm, \
                 tc.tile_pool(name="phdix", bufs=1) as pdix, \
                 tc.tile_pool(name="phdps", bufs=2, space="PSUM") as pdps:
                idx2t = pdix.tile([128, nch * 8], I16)
                nc.sync.dma_start(idx2t[:], idx2[:])
                idxe2t = pdix.tile([128, nch * 8], I16)
                nc.sync.dma_start(idxe2t[:], idxe2[:])
                blkps = None
                for grp in groups:
                    gn = len(grp)
                    g0 = grp[0]
                    gsup = pd.tile([128, gn, R2], BF, tag="g2sup")
                    edg = pd.tile([128, gn, RE], BF, tag="ed2g")
                    r0 = 0
                    while r0 < gn:
                        h = slots[grp[r0]][1]
                        r1 = r0
                        while r1 < gn and slots[grp[r1]][1] == h:
                            r1 += 1
                        tbl2 = (hes2[:] if cfg["nhalves"] == 1
                                else hes2[h * half:h * half + half, :])
                        nidx = (r1 - r0) * 128
                        nc.gpsimd.dma_gather(
                            out_ap=gsup[:, r0:r1, :],
                            in_ap=tbl2,
                            idxs_ap=idx2t[:, (g0 + r0) * 8:(g0 + r1) * 8],
                            num_idxs=nidx, num_idxs_reg=nidx, elem_size=R2)
                        r0 = r1
                    nc.gpsimd.dma_gather(
                        out_ap=edg[:],
                        in_ap=ed2t[:],
                        idxs_ap=idxe2t[:, g0 * 8:(g0 + gn) * 8],
                        num_idxs=gn * 128, num_idxs_reg=gn * 128, elem_size=RE)
                    if strip >= 3:
                        continue
                    es_ap = gsup[:, :, hid:hid + 2].bitcast(F32)
                    ed_ap = edg[:, :, 0:1]
                    lt = pdm.tile([128, gn], F32, tag="lt2")
                    lt3 = _rep(lt[:], [[gn, 128], [1, gn], [1, 1]])
                    nc.vector.tensor_tensor(out=lt3, in0=es_ap, in1=ed_ap,
                                            op=mybir.AluOpType.add)
                    lr = pdm.tile([128, gn], F32, tag="lr2")
                    nc.vector.tensor_scalar(out=lr[:], in0=lt[:],
                                            scalar1=NEG_SLOPE, scalar2=None,
                                            op0=mybir.AluOpType.mult)
                    nc.vector.tensor_tensor(out=lr[:], in0=lt[:], in1=lr[:],
                                            op=mybir.AluOpType.max)
                    ptf = pdm.tile([128, gn], F32, tag="ptf2")
                    nc.scalar.activation(ptf[:], lr[:],
                                         mybir.ActivationFunctionType.Exp)
                    ptb = pdm.tile([128, gn], BF, tag="ptb2")
                    nc.vector.tensor_copy(ptb[:], ptf[:])
                    if strip >= 2:
                        continue
                    for i, ci in enumerate(grp):
                        b, h, first, last = slots[ci]
                        if first:
                            blkps = pdps.tile([128, hid + 1], F32, tag="blkps2")
                        m = pdm.tile([128, 128], BF, tag="m2")
                        nc.vector.tensor_scalar(
                            out=m[:], in0=iota_bf[:],
                            scalar1=dstlt[:, ci:ci + 1], scalar2=None,
                            op0=mybir.AluOpType.is_equal)
                        g2s = _rep(gsup[:], [[gn * R2, 128], [1, hid]],
                                   offset=i * R2)
                        nc.vector.tensor_scalar(
                            out=g2s, in0=g2s,
                            scalar1=ptf[:, i:i + 1], scalar2=None,
                            op0=mybir.AluOpType.mult)
                        nc.tensor.matmul(blkps[:, 0:hid], lhsT=m[:],
                                         rhs=_rep(gsup[:],
                                                  [[gn * R2, 128], [1, hid]],
                                                  offset=i * R2),
                                         start=first, stop=last,
                                         skip_group_check=True)
                        nc.tensor.matmul(blkps[:, hid:hid + 1], lhsT=m[:],
                                         rhs=ptb[:, i:i + 1],
                                         start=False, stop=last,
                                         skip_group_check=True)
                        if last and strip >= 1:
                            continue
                        if last:
                            rs = pdm.tile([128, 1], F32, tag="rs2")
                            nc.vector.reciprocal(rs[:], blkps[:, hid:hid + 1])
                            h2b = pdm.tile([128, hid], BF, tag="h2b")
                            nc.vector.tensor_scalar(
                                out=h2b[:], in0=blkps[:, 0:hid],
                                scalar1=rs[:, 0:1], scalar2=None,
                                op0=mybir.AluOpType.mult)
                            if bias_nonzero[1]:
                                nc.vector.tensor_tensor(
                                    out=h2b[:], in0=h2b[:], in1=b2t[:],
                                    op=mybir.AluOpType.add)
                            ex = pdm.tile([128, hid], BF, tag="ex2")
                            nc.scalar.activation(ex[:], h2b[:],
                                                 mybir.ActivationFunctionType.Exp)
                            nc.vector.tensor_scalar(
                                out=ex[:], in0=ex[:], scalar1=-1.0, scalar2=0.0,
                                op0=mybir.AluOpType.add, op1=mybir.AluOpType.min)
                            nc.vector.tensor_scalar(
                                out=h2b[:], in0=h2b[:], scalar1=0.0, scalar2=None,
                                op0=mybir.AluOpType.max)
                            nc.vector.tensor_tensor(out=h2b[:], in0=h2b[:],
                                                    in1=ex[:],
                                                    op=mybir.AluOpType.add)
                            yt = pdm.tile([128, hid], F32, tag="yt")
                            nc.vector.tensor_tensor(out=yt[:], in0=h2b[:],
                                                    in1=wot[:],
                                                    op=mybir.AluOpType.mult)
                            nc.vector.tensor_reduce(
                                out=yall[:, b:b + 1], in_=yt[:],
                                axis=mybir.AxisListType.X,
                                op=mybir.AluOpType.add)
                if bo_val != 0.0:
                    nc.vector.tensor_scalar(
                        out=yall[:], in0=yall[:], scalar1=float(bo_val),
                        scalar2=None, op0=mybir.AluOpType.add)
                nc.sync.dma_start(yout[:], yall[:])
            if dbg:
                nc.sync.dma_start(h1dbg[:], h1sb[:])
                nc.sync.dma_start(sdbg[:], sdbg_sb[:])
                nc.sync.dma_start(pdbg[:], pdbg_sb[:])
                nc.sync.dma_start(scdbg[:], scdbg_sb[:])

    nc.compile()
    return nc


# ----------------------------------------------------------------------------
# entry point
# ----------------------------------------------------------------------------

_CACHE = {}


def _prepare(x, W1, a_src1, a_dst1, b1, W2, a_src2, a_dst2, b2, Wo, bo,
             edge_index, ncores):
    N, in_ch = x.shape
    heads, hid = a_src1.shape
    HC = heads * hid
    sched = _make_schedule(np.asarray(edge_index), N, ncores, in_ch, heads, hid)
    n_pad, npc = sched["n_pad"], sched["npc"]

    x_pad = np.zeros((n_pad, in_ch), np.float32)
    x_pad[:N] = np.asarray(x, np.float32)

    cols = (np.arange(HC) % heads) * hid + np.arange(HC) // heads
    W1i = np.asarray(W1, np.float32)[:, cols]
    W1r = np.asarray(W1, np.float32).reshape(in_ch, heads, hid)
    w_es1 = np.einsum("chj,hj->ch", W1r, np.asarray(a_src1, np.float32))
    w_ed1 = np.einsum("chj,hj->ch", W1r, np.asarray(a_dst1, np.float32))
    W1ext = np.concatenate([W1i, w_es1, w_ed1], axis=1)

    W2f = np.asarray(W2, np.float32)
    w_es2 = W2f @ np.asarray(a_src2, np.float32)[0]
    w_ed2 = W2f @ np.asarray(a_dst2, np.float32)[0]
    W2ext = np.concatenate([W2f, w_es2[:, None], w_ed2[:, None]], axis=1)
    W2ext = W2ext[cols]  # rows follow interleaved h1 layout
    b1i = np.asarray(b1, np.float32)[cols]

    bfp = lambda a: np.asarray(a, np.float32).astype(np.dtype("bfloat16")
                                                    if False else np.float32)

    def tobf(a):
        import ml_dtypes
        return np.asarray(a, np.float32).astype(ml_dtypes.bfloat16)

    in_maps = []
    for k in range(ncores):
        xTk = tobf(x_pad[sched["perms"][k]].T.copy())
        in_maps.append({
            "xT": xTk,
            "w1e": tobf(W1ext),
            "w2e": tobf(W2ext),
            "b1r": tobf(b1i[None, :]),
            "b2r": tobf(np.asarray(b2, np.float32)[None, :]),
            "wor": tobf(np.asarray(Wo, np.float32).reshape(1, -1)),
            "idx1": sched["idx1"][k],
            "idx2": sched["idx2"][k],
            "idxe1": sched["idxe1"][k],
            "idxe2": sched["idxe2"][k],
            "dstl": sched["dstl"][k],
        })
    bias_nonzero = (bool(np.any(np.asarray(b1) != 0)),
                    bool(np.any(np.asarray(b2) != 0)))
    bo_val = float(np.asarray(bo).reshape(-1)[0])
    meta = dict(sched=sched, heads=heads, hid=hid, in_ch=in_ch,
                bias_nonzero=bias_nonzero, bo_val=bo_val, N=N)
    return meta, in_maps


def _get_program(meta):
    sched = meta["sched"]
    key = (sched["ncores"], sched["n_pad"], sched["nch"],
           tuple(sched["slots"]), meta["bias_nonzero"], meta["bo_val"])
    kh = hash(key)
    if kh not in _CACHE:
        _CACHE[kh] = _build_program(sched, meta["heads"], meta["hid"],
                                    meta["in_ch"], meta["bias_nonzero"],
                                    meta["bo_val"])
    return _CACHE[kh]


def kernel(**inputs):
    ncores = 8
    meta, in_maps = _prepare(
        inputs["x"], inputs["W1"], inputs["a_src1"], inputs["a_dst1"],
        inputs["b1"], inputs["W2"], inputs["a_src2"], inputs["a_dst2"],
        inputs["b2"], inputs["Wo"], inputs["bo"], inputs["edge_index"],
        ncores)
    nc = _get_program(meta)
    res = run_bass_kernel_spmd(nc, in_maps, list(range(ncores)))
    sched = meta["sched"]
    npc, bpc = sched["npc"], sched["bpc"]
    y = np.zeros(sched["n_pad"], np.float32)
    for k in range(ncores):
        yk = res.results[k]["y"]  # [128, bpc]
        y[k * npc:(k + 1) * npc] = yk.T.reshape(npc)
    return y[:meta["N"]].astype(np.float32)



# revision 3
# speedup vs baseline: 1.7310x; 1.7310x over previous
"""2-layer GAT (PyG GATConv semantics) on 8 Trainium2 NeuronCores via Bass/Tile.

Strategy (dst-owner node sharding):
  - Nodes padded to N_pad = 8*npc, core k owns nodes [k*npc, (k+1)*npc).
  - Edges (+self loops) routed to their dst-owner core, grouped by 128-node
    dst block, chunked into 128-edge chunks (half-pure for int16 gather idx).
  - Phase A (replicated): hes1[n] = [x@W1 (c,h interleaved) | es1 f32] for all
    nodes in a per-core permuted order (own nodes first within each table
    half); per-node ed1 extracted to a small gatherable table.
  - Phase B: per chunk, dma_gather 768B rows by src + 256B ed rows by dst,
    p = exp(leakyrelu(es+ed)), one-hot matmul aggregation into PSUM
    (lhsT = M[e,i], rhs = p-scaled h slices + p for softmax denominators),
    per-block normalize + bias + ELU -> h1 (SBUF resident).
  - Phase C: h1 @ W2ext per own block (PE transpose + matmul) -> hes2loc,
    AllGather -> hes2 (global order), ed2 table.
  - Phase D/E: layer-2 edge phase (1 head) + y = h2@Wo + bo per block.
Host does only integer schedule construction, layout packing and weight
folding (a_src/a_dst folded into extra W columns).
"""
import sys

sys.path.insert(0, "/opt/trn_rl_repo")

import contextlib
import dataclasses
import math

import numpy as np

import concourse.bass as bass
import concourse.tile as tile
from concourse import bacc, mybir
from concourse.masks import make_identity
from concourse.bass_utils import run_bass_kernel_spmd

BLK = 128
NEG_SLOPE = 0.2
F32 = mybir.dt.float32
BF = mybir.dt.bfloat16
I16 = mybir.dt.int16
I32 = mybir.dt.int32


def _rep(ap, pattern, offset=None):
    new = dataclasses.replace(ap, ap=pattern)
    if offset is not None:
        new = dataclasses.replace(new, offset=offset)
    return new


# ----------------------------------------------------------------------------
# host-side schedule
# ----------------------------------------------------------------------------

def _make_schedule(edge_index, N, ncores, in_ch, heads, hid):
    bpc = math.ceil(N / (ncores * BLK))
    npc = bpc * BLK
    n_pad = npc * ncores
    half = n_pad // 2

    # per-core permutation: own nodes first within their half; half-preserving
    perms = []
    inv_perms = []
    for k in range(ncores):
        own = np.arange(k * npc, (k + 1) * npc, dtype=np.int64)
        alln = np.arange(n_pad, dtype=np.int64)
        others = alln[(alln < k * npc) | (alln >= (k + 1) * npc)]
        if ncores == 1:
            perm = alln
        elif (k + 1) * npc <= half:  # own range entirely in half A
            a = np.concatenate([own, others[others < half]])
            b = others[others >= half]
            perm = np.concatenate([a, b])
        else:  # own range entirely in half B
            a = others[others < half]
            b = np.concatenate([own, others[others >= half]])
            perm = np.concatenate([a, b])
        inv = np.empty(n_pad, dtype=np.int64)
        inv[perm] = alln
        perms.append(perm)
        inv_perms.append(inv)

    src = np.concatenate([edge_index[0].astype(np.int64),
                          np.arange(n_pad, dtype=np.int64)])
    dst = np.concatenate([edge_index[1].astype(np.int64),
                          np.arange(n_pad, dtype=np.int64)])
    owner = dst // npc

    # per-core edge lists grouped by (block, half)
    per_core = []
    cnt = np.zeros((ncores, bpc, 2), np.int64)
    for k in range(ncores):
        m = owner == k
        s_g = src[m]
        dloc = dst[m] - k * npc
        h = (s_g >= half).astype(np.int64) if ncores > 1 else np.zeros_like(s_g)
        b = dloc // BLK
        order = np.lexsort((h, b))
        s_g, dloc, h, b = s_g[order], dloc[order], h[order], b[order]
        per_core.append((s_g, dloc, h, b))
        np.add.at(cnt[k], (b, h), 1)

    nhalves = 2 if ncores > 1 else 1
    chunks_bh = np.ceil(cnt.max(axis=0) / 128).astype(np.int64)  # [bpc, 2]
    slots = []  # (block, half, first_of_block, last_of_block)
    for b in range(bpc):
        tot = int(chunks_bh[b, :nhalves].sum())
        assert tot >= 1
        i = 0
        for h in range(nhalves):
            for _ in range(int(chunks_bh[b, h])):
                slots.append((b, h, i == 0, i == tot - 1))
                i += 1
    nch = len(slots)

    # per-core packed index arrays
    def wrap16(vals):  # vals [nch, 128] -> [128, nch*8] int16
        v = vals.reshape(nch, 8, 16)
        base = np.zeros((16, nch * 8), np.int16)
        for ci in range(nch):
            base[:, ci * 8:(ci + 1) * 8] = v[ci].T
        out = np.zeros((128, nch * 8), np.int16)
        for r in range(8):
            out[r * 16:(r + 1) * 16] = base
        return out

    idx1 = np.zeros((ncores, 128, nch * 8), np.int16)
    idx2 = np.zeros((ncores, 128, nch * 8), np.int16)
    idxe1 = np.zeros((ncores, 128, nch * 8), np.int16)
    idxe2 = np.zeros((ncores, 128, nch * 8), np.int16)
    dstl = np.full((ncores, 128, nch), -1.0, np.float32)
    for k in range(ncores):
        s_g, dloc, h, b = per_core[k]
        p_src = inv_perms[k][s_g]
        v1 = np.zeros((nch, 128), np.int64)
        v2 = np.zeros((nch, 128), np.int64)
        ve1 = np.zeros((nch, 128), np.int64)
        ve2 = np.zeros((nch, 128), np.int64)
        vd = np.full((nch, 128), -1.0, np.float32)
        # bucket boundaries per (b, h)
        keys = b * 2 + h
        ptr = 0
        ci = 0
        for bb in range(bpc):
            for hh in range(nhalves):
                n = int(cnt[k, bb, hh])
                sl = slice(ptr, ptr + n)
                e_p = p_src[sl] - hh * half
                e_g = s_g[sl] - hh * half
                e_d = dloc[sl] + (npc if (ncores > 1 and k >= ncores // 2) else 0)
                e_dl = (dloc[sl] - bb * BLK).astype(np.float32)
                e_d2 = dloc[sl]
                ptr += n
                ncb = int(chunks_bh[bb, hh])
                for c in range(ncb):
                    a0, a1 = c * 128, min((c + 1) * 128, n)
                    ln = max(0, a1 - a0)
                    if ln > 0:
                        v1[ci, :ln] = e_p[a0:a0 + ln]
                        v2[ci, :ln] = e_g[a0:a0 + ln]
                        ve1[ci, :ln] = e_d[a0:a0 + ln]
                        ve2[ci, :ln] = e_d2[a0:a0 + ln]
                        vd[ci, :ln] = e_dl[a0:a0 + ln]
                    ci += 1
        assert ci == nch and ptr == len(s_g)
        idx1[k] = wrap16(v1)
        idx2[k] = wrap16(v2)
        idxe1[k] = wrap16(ve1)
        idxe2[k] = wrap16(ve2)
        dstl[k] = vd.T

    return dict(
        ncores=ncores, bpc=bpc, npc=npc, n_pad=n_pad, half=half,
        nhalves=nhalves, slots=slots, nch=nch,
        perms=perms, idx1=idx1, idx2=idx2, idxe1=idxe1, idxe2=idxe2,
        dstl=dstl,
    )


# ----------------------------------------------------------------------------
# device program
# ----------------------------------------------------------------------------

def _build_program(cfg, heads, hid, in_ch, bias_nonzero, bo_val, dbg=False,
                   sim_nocc=False, lim_blocks=None, strip=0, reps=1):
    ncores = cfg["ncores"]
    bpc, npc, n_pad, half = cfg["bpc"], cfg["npc"], cfg["n_pad"], cfg["half"]
    slots, nch = cfg["slots"], cfg["nch"]
    HC = heads * hid  # 256
    R1 = 384           # hes1 row elems (bf16): h 0:256, es f32 slots 256:264
    R2 = 128           # hes2 row elems: h2 0:64, es2 f32 slots 64:66
    RE = 128           # ed table row elems
    W1C = HC + 2 * heads  # 264
    W2C = hid + 2          # 66
    ntile = n_pad // BLK

    nc = bacc.Bacc("TRN2", target_bir_lowering=False, debug=False,
                   enable_asserts=True, num_devices=ncores,
                   num_swdge_queues=4, dynamic_dma_scratch_size=65536)
    _qn = [0]

    def _next_q():
        q = _qn[0]
        _qn[0] = (q + 1) % 4
        return q

    xT = nc.dram_tensor("xT", [in_ch, n_pad], BF, kind="ExternalInput").ap()
    w1e = nc.dram_tensor("w1e", [in_ch, W1C], BF, kind="ExternalInput").ap()
    w2e = nc.dram_tensor("w2e", [HC, W2C], BF, kind="ExternalInput").ap()
    b1r = nc.dram_tensor("b1r", [1, HC], BF, kind="ExternalInput").ap()
    b2r = nc.dram_tensor("b2r", [1, hid], BF, kind="ExternalInput").ap()
    wor = nc.dram_tensor("wor", [1, hid], BF, kind="ExternalInput").ap()
    idx1 = nc.dram_tensor("idx1", [128, nch * 8], I16, kind="ExternalInput").ap()
    idx2 = nc.dram_tensor("idx2", [128, nch * 8], I16, kind="ExternalInput").ap()
    idxe1 = nc.dram_tensor("idxe1", [128, nch * 8], I16, kind="ExternalInput").ap()
    idxe2 = nc.dram_tensor("idxe2", [128, nch * 8], I16, kind="ExternalInput").ap()
    dstl = nc.dram_tensor("dstl", [128, nch], F32, kind="ExternalInput").ap()
    yout = nc.dram_tensor("y", [128, bpc], F32, kind="ExternalOutput").ap()

    hes1 = nc.dram_tensor("hes1", [n_pad, R1], BF).ap()
    h1dbg = (nc.dram_tensor("h1dbg", [128, bpc * HC], BF,
                            kind="ExternalOutput").ap() if dbg else None)
    sdbg = (nc.dram_tensor("sdbg", [128, bpc * heads], F32,
                           kind="ExternalOutput").ap() if dbg else None)
    aggdbg = (nc.dram_tensor("aggdbg", [128, HC], F32,
                             kind="ExternalOutput").ap() if dbg else None)
    pdbg = (nc.dram_tensor("pdbg", [128, 64], F32,
                           kind="ExternalOutput").ap() if dbg else None)
    scdbg = (nc.dram_tensor("scdbg", [128, HC], F32,
                            kind="ExternalOutput").ap() if dbg else None)
    ned1 = npc * (2 if ncores > 1 else 1)
    ed1t = nc.dram_tensor("ed1t", [ned1, RE], BF).ap()
    ed2t = nc.dram_tensor("ed2t", [npc, RE], BF).ap()
    hes2loc = nc.dram_tensor("hes2loc", [npc, R2], BF).ap()
    if ncores > 1:
        hes2 = nc.dram_tensor("hes2", [n_pad, R2], BF,
                              addr_space="Shared").ap()
    else:
        hes2 = nc.dram_tensor("hes2", [n_pad, R2], BF).ap()

    # own tile ranges in phase A (permuted order)
    if ncores == 1:
        own_ranges = [(0, bpc, 0)]
    else:
        own_ranges = [(0, bpc, 0), (half // BLK, half // BLK + bpc, npc)]

    def own_row_off(t):
        for lo, hi, off in own_ranges:
            if lo <= t < hi:
                return off + (t - lo) * BLK
        return None

    with tile.TileContext(nc) as tc, contextlib.ExitStack() as _stk:
        consts = _stk.enter_context(tc.tile_pool(name="consts", bufs=1))
        iota_i = consts.tile([128, 128], I32)
        nc.gpsimd.iota(iota_i[:], pattern=[[1, 128]], base=0, channel_multiplier=0)
        iota_bf = consts.tile([128, 128], BF)
        nc.vector.tensor_copy(iota_bf[:], iota_i[:])
        ident = consts.tile([128, 128], BF)
        make_identity(nc, ident[:])
        w1t = consts.tile([128, W1C], BF)
        nc.sync.dma_start(w1t[:], w1e[:])
        w2t = consts.tile([128, 2, W2C], BF)
        nc.sync.dma_start(w2t[:, 0, :], w2e[0:128, :])
        nc.sync.dma_start(w2t[:, 1, :], w2e[128:256, :])
        b1t = consts.tile([128, HC], BF)
        b2t = consts.tile([128, hid], BF)
        wot = consts.tile([128, hid], BF)
        onecol = consts.tile([1, 128], BF)
        nc.gpsimd.memset(onecol[:], 1.0)
        b1row = consts.tile([1, HC], BF)
        nc.sync.dma_start(b1row[:], b1r[:])
        b2row = consts.tile([1, hid], BF)
        nc.sync.dma_start(b2row[:], b2r[:])
        worow = consts.tile([1, hid], BF)
        nc.sync.dma_start(worow[:], wor[:])
        idx1t = consts.tile([128, nch * 8], I16)
        nc.sync.dma_start(idx1t[:], idx1[:])
        idxe1t = consts.tile([128, nch * 8], I16)
        nc.sync.dma_start(idxe1t[:], idxe1[:])
        dstlt = consts.tile([128, nch], F32)
        nc.sync.dma_start(dstlt[:], dstl[:])
        h1sb = consts.tile([128, bpc * HC], BF)
        ed2sb = consts.tile([128, bpc], BF)
        yall = consts.tile([128, bpc], F32)
        if lim_blocks is not None or strip > 0:
            nc.vector.memset(yall[:], 0.0)
            nc.vector.memset(h1sb[:], 0.0)
        if dbg:
            sdbg_sb = consts.tile([128, bpc * heads], F32)
            pdbg_sb = consts.tile([128, 64], F32)
            scdbg_sb = consts.tile([128, HC], F32)

        with tc.tile_pool(name="bcp", bufs=1, space="PSUM") as bcp:
            bps = bcp.tile([128, HC], F32)
            nc.tensor.matmul(bps[:, 0:HC], lhsT=onecol[:], rhs=b1row[:],
                             start=True, stop=True)
            nc.vector.tensor_copy(b1t[:], bps[:, 0:HC])
            bps2 = bcp.tile([128, HC], F32, tag="bps")
            nc.tensor.matmul(bps2[:, 0:hid], lhsT=onecol[:], rhs=b2row[:],
                             start=True, stop=True)
            nc.vector.tensor_copy(b2t[:], bps2[:, 0:hid])
            bps3 = bcp.tile([128, HC], F32, tag="bps")
            nc.tensor.matmul(bps3[:, 0:hid], lhsT=onecol[:], rhs=worow[:],
                             start=True, stop=True)
            nc.vector.tensor_copy(wot[:], bps3[:, 0:hid])

        # ---------------- phase A: hes1 = [x@W1 | es1], ed1 ----------------
        AT = 4
        RW = HC + 2 * heads
        with tc.tile_pool(name="pha", bufs=3) as pa, \
             tc.tile_pool(name="phaps", bufs=4, space="PSUM") as paps, \
             tc.tile_pool(name="phaed", bufs=2) as paed:
            for t0 in range(0, ntile, AT):
                tn = min(AT, ntile - t0)
                xt = pa.tile([128, tn * BLK], BF, tag="xt")
                nc.sync.dma_start(xt[:], xT[:, t0 * BLK:(t0 + tn) * BLK])
                row4 = pa.tile([128, tn, RW], BF, tag="row")
                for j in range(tn):
                    t = t0 + j
                    ps = paps.tile([128, W1C], F32, tag="ps")
                    nc.tensor.matmul(ps[:], lhsT=xt[:, j * BLK:(j + 1) * BLK],
                                     rhs=w1t[:], start=True, stop=True)
                    nc.scalar.copy(
                        _rep(row4[:], [[tn * RW, 128], [1, HC]], offset=j * RW),
                        ps[:, 0:HC])
                    nc.vector.tensor_copy(
                        _rep(row4[:], [[tn * RW, 128], [1, 2 * heads]],
                             offset=j * RW + HC).bitcast(F32),
                        ps[:, HC:HC + heads])
                    off = own_row_off(t)
                    if off is not None:
                        eds = paed.tile([128, RE], BF, tag="eds")
                        nc.vector.memset(eds[:], 0.0)
                        nc.vector.tensor_copy(eds[:, 0:heads],
                                              ps[:, HC + heads:HC + 2 * heads])
                        nc.sync.dma_start(ed1t[off:off + BLK, :], eds[:])
                out_ap = _rep(hes1[:],
                              [[R1, BLK], [BLK * R1, tn], [1, RW]],
                              offset=t0 * BLK * R1)
                nc.sync.dma_start(out_ap, row4[:])

        for _repeat_i in range(reps):
            # ---------------- phases B+C per own block ----------------
            GG = 8
            lim = bpc if lim_blocks is None else lim_blocks
            used = [ci for ci in range(nch) if slots[ci][0] < lim]
            groups = [used[g0:g0 + GG] for g0 in range(0, len(used), GG)]

            with tc.tile_pool(name="phb", bufs=4) as pb, \
                 tc.tile_pool(name="phbm", bufs=4) as pbm, \
                 tc.tile_pool(name="phbps", bufs=2, space="PSUM") as pbps, \
                 tc.tile_pool(name="phc", bufs=2) as pc, \
                 tc.tile_pool(name="phcps", bufs=2, space="PSUM") as pcps:
                blkps = None
                for grp in groups:
                    gn = len(grp)
                    g0 = grp[0]
                    assert grp == list(range(g0, g0 + gn))
                    gsup = pb.tile([128, gn, R1], BF, tag="gsup")
                    edg = pb.tile([128, gn, RE], BF, tag="edg")
                    # half-pure sub-ranges within the group share one gather
                    r0 = 0
                    while r0 < gn:
                        h = slots[grp[r0]][1]
                        r1 = r0
                        while r1 < gn and slots[grp[r1]][1] == h:
                            r1 += 1
                        tbl1 = (hes1[:] if cfg["nhalves"] == 1
                                else hes1[h * half:h * half + half, :])
                        nidx = (r1 - r0) * 128
                        nc.gpsimd.dma_gather(
                            out_ap=gsup[:, r0:r1, :],
                            in_ap=tbl1,
                            idxs_ap=idx1t[:, (g0 + r0) * 8:(g0 + r1) * 8],
                            num_idxs=nidx, num_idxs_reg=nidx, elem_size=R1,
                            queue_num=_next_q())
                        r0 = r1
                    nc.gpsimd.dma_gather(
                        out_ap=edg[:],
                        in_ap=ed1t[:],
                        idxs_ap=idxe1t[:, g0 * 8:(g0 + gn) * 8],
                        num_idxs=gn * 128, num_idxs_reg=gn * 128, elem_size=RE,
                        queue_num=_next_q())
                    if strip >= 3:
                        continue
                    # p = exp(lrelu(es + ed)) for the whole group
                    es_ap = gsup[:, :, HC:HC + 2 * heads].bitcast(F32)
                    ed_ap = edg[:, :, 0:heads]
                    lt = pbm.tile([128, gn * heads], F32, tag="lt")
                    lt3 = _rep(lt[:], [[gn * heads, 128], [heads, gn], [1, heads]])
                    nc.vector.tensor_tensor(out=lt3, in0=es_ap, in1=ed_ap,
                                            op=mybir.AluOpType.add)
                    lr = pbm.tile([128, gn * heads], F32, tag="lr")
                    nc.vector.tensor_scalar(out=lr[:], in0=lt[:],
                                            scalar1=NEG_SLOPE, scalar2=None,
                                            op0=mybir.AluOpType.mult)
                    nc.vector.tensor_tensor(out=lr[:], in0=lt[:], in1=lr[:],
                                            op=mybir.AluOpType.max)
                    ptf = pbm.tile([128, gn * heads], F32, tag="ptf")
                    nc.scalar.activation(ptf[:], lr[:],
                                         mybir.ActivationFunctionType.Exp)
                    ptb = pbm.tile([128, gn * heads], BF, tag="ptb")
                    nc.vector.tensor_copy(ptb[:], ptf[:])
                    if dbg and grp[0] == 0:
                        nc.vector.tensor_copy(pdbg_sb[:, 0:gn * heads], ptf[:])
                    if strip >= 2:
                        continue
                    for i, ci in enumerate(grp):
                        b, h, first, last = slots[ci]
                        if first:
                            blkps = pbps.tile([128, HC + heads], F32, tag="blkps")
                        m = pbm.tile([128, 128], BF, tag="m")
                        nc.vector.tensor_scalar(
                            out=m[:], in0=iota_bf[:],
                            scalar1=dstlt[:, ci:ci + 1], scalar2=None,
                            op0=mybir.AluOpType.is_equal)
                        # scale gathered h rows by p (interleaved (c,h) layout)
                        gslice = _rep(gsup[:],
                                      [[gn * R1, 128], [heads, hid], [1, heads]],
                                      offset=i * R1)
                        pbc = _rep(ptb[:], [[gn * heads, 128], [0, hid], [1, heads]],
                                   offset=i * heads)
                        nc.vector.tensor_tensor(out=gslice, in0=gslice, in1=pbc,
                                                op=mybir.AluOpType.mult)
                        if dbg and ci == 0:
                            nc.vector.tensor_copy(
                                scdbg_sb[:],
                                _rep(gsup[:], [[gn * R1, 128], [1, HC]], offset=0))
                        rhs = _rep(gsup[:], [[gn * R1, 128], [1, HC]],
                                   offset=i * R1)
                        nc.tensor.matmul(
                            blkps[:, 0:HC], lhsT=m[:], rhs=rhs,
                            start=first, stop=last, skip_group_check=True)
                        nc.tensor.matmul(
                            blkps[:, HC:HC + heads], lhsT=m[:],
                            rhs=ptb[:, i * heads:(i + 1) * heads],
                            start=False, stop=last, skip_group_check=True)

                        if last and strip >= 1:
                            continue
                        if last:
                            # -------- block drain: normalize, bias, ELU --------
                            if dbg:
                                nc.vector.tensor_copy(
                                    sdbg_sb[:, b * heads:(b + 1) * heads],
                                    blkps[:, HC:HC + heads])
                                if b == 0:
                                    nc.vector.tensor_copy(pdbg_sb[:, 32:64],
                                                          blkps[:, 0:32])
                            rs = pbm.tile([128, heads], F32, tag="rs")
                            nc.vector.reciprocal(rs[:], blkps[:, HC:HC + heads])
                            h1b = pbm.tile([128, HC], BF, tag="h1b")
                            ps3 = _rep(blkps[:],
                                       [[HC + heads, 128], [heads, hid], [1, heads]])
                            rsb = _rep(rs[:], [[heads, 128], [0, hid], [1, heads]])
                            h1b3 = _rep(h1b[:], [[HC, 128], [heads, hid], [1, heads]])
                            nc.vector.tensor_tensor(out=h1b3, in0=ps3, in1=rsb,
                                                    op=mybir.AluOpType.mult)
                            if bias_nonzero[0]:
                                nc.vector.tensor_tensor(
                                    out=h1b[:], in0=h1b[:], in1=b1t[:],
                                    op=mybir.AluOpType.add)
                            # elu = max(x,0) + min(exp(x)-1, 0)
                            ex = pbm.tile([128, HC], BF, tag="ex")
                            nc.scalar.activation(ex[:], h1b[:],
                                                 mybir.ActivationFunctionType.Exp)
                            nc.vector.tensor_scalar(
                                out=ex[:], in0=ex[:], scalar1=-1.0, scalar2=0.0,
                                op0=mybir.AluOpType.add, op1=mybir.AluOpType.min)
                            h1f = h1sb[:, b * HC:(b + 1) * HC]
                            nc.vector.tensor_scalar(
                                out=h1f, in0=h1b[:], scalar1=0.0, scalar2=None,
                                op0=mybir.AluOpType.max)
                            nc.vector.tensor_tensor(out=h1f, in0=h1f, in1=ex[:],
                                                    op=mybir.AluOpType.add)

                            # -------- phase C for this block --------
                            if strip == 4:
                                continue
                            ps2 = pcps.tile([128, W2C], F32, tag="ps2")
                            for kk in range(2):
                                trp = pcps.tile([128, 128], BF, tag="trp")
                                nc.tensor.transpose(
                                    trp[:],
                                    h1sb[:, b * HC + kk * 128: b * HC + kk * 128 + 128],
                                    ident[:])
                                h1T = pc.tile([128, 128], BF, tag="h1T")
                                nc.vector.tensor_copy(h1T[:], trp[:])
                                nc.tensor.matmul(ps2[:], lhsT=h1T[:],
                                                 rhs=w2t[:, kk, :],
                                                 start=kk == 0, stop=kk == 1)
                            h2row = pc.tile([128, R2], BF, tag="h2row")
                            nc.vector.memset(h2row[:], 0.0)
                            nc.vector.tensor_copy(h2row[:, 0:hid], ps2[:, 0:hid])
                            nc.vector.tensor_copy(
                                h2row[:, hid:hid + 2].bitcast(F32),
                                ps2[:, hid:hid + 1])
                            nc.sync.dma_start(hes2loc[b * BLK:(b + 1) * BLK, :],
                                              h2row[:])
                            ed2s = pc.tile([128, RE], BF, tag="ed2s")
                            nc.vector.memset(ed2s[:], 0.0)
                            nc.vector.tensor_copy(ed2s[:, 0:1],
                                                  ps2[:, hid + 1:hid + 2])
                            nc.sync.dma_start(ed2t[b * BLK:(b + 1) * BLK, :],
                                              ed2s[:])

            # ---------------- AllGather ----------------
            if strip >= 1:
                nc.vector.memset(h1sb[:], 0.0)
            if ncores > 1 and not sim_nocc:
                nc.gpsimd.collective_compute(
                    "AllGather", mybir.AluOpType.bypass,
                    replica_groups=[list(range(ncores))],
                    ins=[hes2loc[:]], outs=[hes2[:]])
            elif ncores == 1:
                nc.sync.dma_start(hes2[:], hes2loc[:])
            else:  # sim_nocc stand-in for AllGather: same local write volume
                for r in range(ncores):
                    nc.sync.dma_start(hes2[r * npc:(r + 1) * npc, :], hes2loc[:])

            # ---------------- phases D+E ----------------
            with tc.tile_pool(name="phd", bufs=4) as pd, \
                 tc.tile_pool(name="phdm", bufs=4) as pdm, \
                 tc.tile_pool(name="phdix", bufs=1) as pdix, \
                 tc.tile_pool(name="phdps", bufs=2, space="PSUM") as pdps:
                idx2t = pdix.tile([128, nch * 8], I16)
                nc.sync.dma_start(idx2t[:], idx2[:])
                idxe2t = pdix.tile([128, nch * 8], I16)
                nc.sync.dma_start(idxe2t[:], idxe2[:])
                blkps = None
                for grp in groups:
                    gn = len(grp)
                    g0 = grp[0]
                    gsup = pd.tile([128, gn, R2], BF, tag="g2sup")
                    edg = pd.tile([128, gn, RE], BF, tag="ed2g")
                    r0 = 0
                    while r0 < gn:
                        h = slots[grp[r0]][1]
                        r1 = r0
                        while r1 < gn and slots[grp[r1]][1] == h:
                            r1 += 1
                        tbl2 = (hes2[:] if cfg["nhalves"] == 1
                                else hes2[h * half:h * half + half, :])
                        nidx = (r1 - r0) * 128
                        nc.gpsimd.dma_gather(
                            out_ap=gsup[:, r0:r1, :],
                            in_ap=tbl2,
                            idxs_ap=idx2t[:, (g0 + r0) * 8:(g0 + r1) * 8],
                            num_idxs=nidx, num_idxs_reg=nidx, elem_size=R2,
                            queue_num=_next_q())
                        r0 = r1
                    nc.gpsimd.dma_gather(
                        out_ap=edg[:],
                        in_ap=ed2t[:],
                        idxs_ap=idxe2t[:, g0 * 8:(g0 + gn) * 8],
                        num_idxs=gn * 128, num_idxs_reg=gn * 128, elem_size=RE,
                        queue_num=_next_q())
                    if strip >= 3:
                        continue
                    es_ap = gsup[:, :, hid:hid + 2].bitcast(F32)
                    ed_ap = edg[:, :, 0:1]
                    lt = pdm.tile([128, gn], F32, tag="lt2")
                    lt3 = _rep(lt[:], [[gn, 128], [1, gn], [1, 1]])
                    nc.vector.tensor_tensor(out=lt3, in0=es_ap, in1=ed_ap,
                                            op=mybir.AluOpType.add)
                    lr = pdm.tile([128, gn], F32, tag="lr2")
                    nc.vector.tensor_scalar(out=lr[:], in0=lt[:],
                                            scalar1=NEG_SLOPE, scalar2=None,
                                            op0=mybir.AluOpType.mult)
                    nc.vector.tensor_tensor(out=lr[:], in0=lt[:], in1=lr[:],
                                            op=mybir.AluOpType.max)
                    ptf = pdm.tile([128, gn], F32, tag="ptf2")
                    nc.scalar.activation(ptf[:], lr[:],
                                         mybir.ActivationFunctionType.Exp)
                    ptb = pdm.tile([128, gn], BF, tag="ptb2")
                    nc.vector.tensor_copy(ptb[:], ptf[:])
                    if strip >= 2:
                        continue
                    for i, ci in enumerate(grp):
                        b, h, first, last = slots[ci]
                        if first:
                            blkps = pdps.tile([128, hid + 1], F32, tag="blkps2")
                        m = pdm.tile([128, 128], BF, tag="m2")
                        nc.vector.tensor_scalar(
                            out=m[:], in0=iota_bf[:],
                            scalar1=dstlt[:, ci:ci + 1], scalar2=None,
                            op0=mybir.AluOpType.is_equal)
                        g2s = _rep(gsup[:], [[gn * R2, 128], [1, hid]],
                                   offset=i * R2)
                        nc.vector.tensor_scalar(
                            out=g2s, in0=g2s,
                            scalar1=ptf[:, i:i + 1], scalar2=None,
                            op0=mybir.AluOpType.mult)
                        nc.tensor.matmul(blkps[:, 0:hid], lhsT=m[:],
                                         rhs=_rep(gsup[:],
                                                  [[gn * R2, 128], [1, hid]],
                                                  offset=i * R2),
                                         start=first, stop=last,
                                         skip_group_check=True)
                        nc.tensor.matmul(blkps[:, hid:hid + 1], lhsT=m[:],
                                         rhs=ptb[:, i:i + 1],
                                         start=False, stop=last,
                                         skip_group_check=True)
                        if last and strip >= 1:
                            continue
                        if last:
                            rs = pdm.tile([128, 1], F32, tag="rs2")
                            nc.vector.reciprocal(rs[:], blkps[:, hid:hid + 1])
                            h2b = pdm.tile([128, hid], BF, tag="h2b")
                            nc.vector.tensor_scalar(
                                out=h2b[:], in0=blkps[:, 0:hid],
                                scalar1=rs[:, 0:1], scalar2=None,
                                op0=mybir.AluOpType.mult)
                            if bias_nonzero[1]:
                                nc.vector.tensor_tensor(
                                    out=h2b[:], in0=h2b[:], in1=b2t[:],
                                    op=mybir.AluOpType.add)
                            ex = pdm.tile([128, hid], BF, tag="ex2")
                            nc.scalar.activation(ex[:], h2b[:],
                                                 mybir.ActivationFunctionType.Exp)
                            nc.vector.tensor_scalar(
                                out=ex[:], in0=ex[:], scalar1=-1.0, scalar2=0.0,
                                op0=mybir.AluOpType.add, op1=mybir.AluOpType.min)
                            nc.vector.tensor_scalar(
                                out=h2b[:], in0=h2b[:], scalar1=0.0, scalar2=None,
                                op0=mybir.AluOpType.max)
                            nc.vector.tensor_tensor(out=h2b[:], in0=h2b[:],
                                                    in1=ex[:],
                                                    op=mybir.AluOpType.add)
                            yt = pdm.tile([128, hid], F32, tag="yt")
                            nc.vector.tensor_tensor(out=yt[:], in0=h2b[:],
                                                    in1=wot[:],
                                                    op=mybir.AluOpType.mult)
                            nc.vector.tensor_reduce(
                                out=yall[:, b:b + 1], in_=yt[:],
                                axis=mybir.AxisListType.X,
                                op=mybir.AluOpType.add)
                if bo_val != 0.0:
                    nc.vector.tensor_scalar(
                        out=yall[:], in0=yall[:], scalar1=float(bo_val),
                        scalar2=None, op0=mybir.AluOpType.add)
                nc.sync.dma_start(yout[:], yall[:])
            if dbg:
                nc.sync.dma_start(h1dbg[:], h1sb[:])
                nc.sync.dma_start(sdbg[:], sdbg_sb[:])
                nc.sync.dma_start(pdbg[:], pdbg_sb[:])
                nc.sync.dma_start(scdbg[:], scdbg_sb[:])

    nc.compile()
    return nc


# ----------------------------------------------------------------------------
# entry point
# ----------------------------------------------------------------------------

_CACHE = {}


def _prepare(x, W1, a_src1, a_dst1, b1, W2, a_src2, a_dst2, b2, Wo, bo,
             edge_index, ncores):
    N, in_ch = x.shape
    heads, hid = a_src1.shape
    HC = heads * hid
    sched = _make_schedule(np.asarray(edge_index), N, ncores, in_ch, heads, hid)
    n_pad, npc = sched["n_pad"], sched["npc"]

    x_pad = np.zeros((n_pad, in_ch), np.float32)
    x_pad[:N] = np.asarray(x, np.float32)

    cols = (np.arange(HC) % heads) * hid + np.arange(HC) // heads
    W1i = np.asarray(W1, np.float32)[:, cols]
    W1r = np.asarray(W1, np.float32).reshape(in_ch, heads, hid)
    w_es1 = np.einsum("chj,hj->ch", W1r, np.asarray(a_src1, np.float32))
    w_ed1 = np.einsum("chj,hj->ch", W1r, np.asarray(a_dst1, np.float32))
    W1ext = np.concatenate([W1i, w_es1, w_ed1], axis=1)

    W2f = np.asarray(W2, np.float32)
    w_es2 = W2f @ np.asarray(a_src2, np.float32)[0]
    w_ed2 = W2f @ np.asarray(a_dst2, np.float32)[0]
    W2ext = np.concatenate([W2f, w_es2[:, None], w_ed2[:, None]], axis=1)
    W2ext = W2ext[cols]  # rows follow interleaved h1 layout
    b1i = np.asarray(b1, np.float32)[cols]

    bfp = lambda a: np.asarray(a, np.float32).astype(np.dtype("bfloat16")
                                                    if False else np.float32)

    def tobf(a):
        import ml_dtypes
        return np.asarray(a, np.float32).astype(ml_dtypes.bfloat16)

    in_maps = []
    for k in range(ncores):
        xTk = tobf(x_pad[sched["perms"][k]].T.copy())
        in_maps.append({
            "xT": xTk,
            "w1e": tobf(W1ext),
            "w2e": tobf(W2ext),
            "b1r": tobf(b1i[None, :]),
            "b2r": tobf(np.asarray(b2, np.float32)[None, :]),
            "wor": tobf(np.asarray(Wo, np.float32).reshape(1, -1)),
            "idx1": sched["idx1"][k],
            "idx2": sched["idx2"][k],
            "idxe1": sched["idxe1"][k],
            "idxe2": sched["idxe2"][k],
            "dstl": sched["dstl"][k],
        })
    bias_nonzero = (bool(np.any(np.asarray(b1) != 0)),
                    bool(np.any(np.asarray(b2) != 0)))
    bo_val = float(np.asarray(bo).reshape(-1)[0])
    meta = dict(sched=sched, heads=heads, hid=hid, in_ch=in_ch,
                bias_nonzero=bias_nonzero, bo_val=bo_val, N=N)
    return meta, in_maps


def _get_program(meta):
    sched = meta["sched"]
    key = (sched["ncores"], sched["n_pad"], sched["nch"],
           tuple(sched["slots"]), meta["bias_nonzero"], meta["bo_val"])
    kh = hash(key)
    if kh not in _CACHE:
        _CACHE[kh] = _build_program(sched, meta["heads"], meta["hid"],
                                    meta["in_ch"], meta["bias_nonzero"],
                                    meta["bo_val"])
    return _CACHE[kh]


def kernel(**inputs):
    ncores = 8
    meta, in_maps = _prepare(
        inputs["x"], inputs["W1"], inputs["a_src1"], inputs["a_dst1"],
        inputs["b1"], inputs["W2"], inputs["a_src2"], inputs["a_dst2"],
        inputs["b2"], inputs["Wo"], inputs["bo"], inputs["edge_index"],
        ncores)
    nc = _get_program(meta)
    res = run_bass_kernel_spmd(nc, in_maps, list(range(ncores)))
    sched = meta["sched"]
    npc, bpc = sched["npc"], sched["bpc"]
    y = np.zeros(sched["n_pad"], np.float32)
    for k in range(ncores):
        yk = res.results[k]["y"]  # [128, bpc]
        y[k * npc:(k + 1) * npc] = yk.T.reshape(npc)
    return y[:meta["N"]].astype(np.float32)

